# revision 30
# baseline (speedup 1.0000x reference)
"""Trainium2 Bass kernel for nn_BalancedHamiltonLayer.

The reference computes, per token-matrix X_n (32x32 view of each 1024-dim
token):  out_n = sum_r H_r @ X_n @ B_r^T  + bias, with H_r the 32x32 Hamilton
matrix of A_stack[r].  That is a fixed linear map on the flattened token:

    out[n, k*32+j] = sum_{s,i} W[s*32+i, k*32+j] * x[n, s*32+i]
    W[si, kj]      = sum_r H[r,k,s] * B[r,j,i]          (1024x1024, fp32)

so the whole layer is one dense (12288, 1024) @ (1024, 1024) matmul + bias.
The tiny factor stacks are folded on the host; x is sharded along the batch
axis across the 8 NeuronCores (data parallel, no collectives).

W's 4x4 grid of 256x256 blocks are +-copies of only FOUR unique matrices
M_q[(sr,i),(kr,j)] = sum_r A[r,q,kr,sr]*B[r,j,i] (quaternion structure), so
the host ships 2 MB of +-M instead of 4 MB of W, and every matmul rhs is a
(128, 256) view straight into that pack - no on-chip weight expansion.

Per core (1536 tokens, 12 blocks of 128), software-pipelined:
  natural-layout x DMA -> PE transpose (fp32r, via identity) into PSUM ->
  one DVE evacuation per block -> 32 fp32r matmuls (K=8x128, N=4x256)
  accumulating in PSUM -> DVE bias-add -> DMA out (scalar-engine HWDGE).
Transposes are emitted one block ahead of the matmuls so the DVE evacuation
overlaps the previous block's matmuls instead of stalling the PE.

PSUM bank accumulation: only the FIRST matmul touching a bank per block uses
start=True (which clears the whole bank's has_written bits); the second
256-column slice then overwrites-where-clear, and all k>0 matmuls
accumulate.  fp32r (rounded fp32) streams at 1 cycle/row for N>=256 (4x
faster than fp32) at ~1.5e-4 relative error.

Self-loading 4-byte Matmults only fit ONE sync wait in the S3_LW ISA
struct, so the kernel keeps every Matmult at <=1 wait: PE warm-up
transposes (interleaved with block 0's matmuls) absorb the weight-pack DMA
deps, and all PSUM evacuations go through the vector engine so a single
DVE-sem wait on the first matmul of each block transitively covers every
WAR hazard.
"""

import numpy as np

B, T, D = 48, 256, 1024
N_CORES = 8
TOK = B * T                     # 12288 tokens
TOK_PER_CORE = TOK // N_CORES   # 1536
BLOCKS = TOK_PER_CORE // 128    # 12
KT = D // 128                   # 8 contraction tiles

# Quaternion block structure: W block (sb, kb) = SGN[kb][sb] * M[Q[kb][sb]]
Q_IDX = [[0, 1, 2, 3], [1, 0, 3, 2], [2, 3, 0, 1], [3, 2, 1, 0]]
SGN = [[1, -1, -1, -1], [1, 1, -1, 1], [1, 1, 1, -1], [1, -1, 1, 1]]

_cached_nc = None


def _build():
    import concourse.bacc as bacc
    import concourse.mybir as mybir
    import concourse.tile as tile

    F32R = mybir.dt.float32r
    F32 = mybir.dt.float32

    nc = bacc.Bacc("TRN2", target_bir_lowering=False)
    x_d = nc.dram_tensor("x", [TOK_PER_CORE, D], F32R, kind="ExternalInput")
    mp_d = nc.dram_tensor("mpack", [128, 2, 4, 256], F32R, kind="ExternalInput")
    b_d = nc.dram_tensor("biasb", [1, D], F32, kind="ExternalInput")
    o_d = nc.dram_tensor("out", [TOK_PER_CORE, D], F32, kind="ExternalOutput")

    with tile.TileContext(nc) as tc:
        with (
            tc.tile_pool(name="consts", bufs=1) as consts,
            tc.tile_pool(name="xin", bufs=6) as xin_pool,
            tc.tile_pool(name="xt", bufs=6) as xt_pool,
            tc.tile_pool(name="outp", bufs=BLOCKS) as out_pool,
            tc.tile_pool(name="psum_t", bufs=2, space="PSUM") as psum_t,
            tc.tile_pool(name="psum_mm", bufs=4, space="PSUM") as psum_mm,
        ):
            # identity built on-chip: gpsimd writes f32, DVE cast-copies to
            # f32r (the canonical "round to fp32r" producer) - no DMA.
            from concourse.masks import make_identity

            identity_f32 = consts.tile([128, 128], F32)
            make_identity(nc, identity_f32)
            identity = consts.tile([128, 128], F32R)
            nc.vector.tensor_copy(out=identity[:], in_=identity_f32[:])

            m_sb = consts.tile([128, 2, 2, 4, 256], F32R)
            # Prefetch the first x blocks ahead of the weight pack so the PE
            # has transpose work while the weights stream in.
            early_x = {}
            for blk in range(2):
                x_sb = xin_pool.tile([128, D], F32R, tag="x_sb", name=f"x_sb_{blk}")
                if blk == 0:
                    nc.sync.dma_start(x_sb[:, 0:512], x_d[0:128, 0:512])
                    nc.sync.dma_start(x_sb[:, 512:D], x_d[0:128, 512:D])
                else:
                    nc.sync.dma_start(x_sb[:], x_d[blk * 128 : (blk + 1) * 128, :])
                early_x[blk] = x_sb


            # Weight pack DMAs: +M only (1 MB); -M is produced on the DVE
            # during block 0 (cheaper than 1 MB more of front-loaded DMA).
            for h in (0, 1):
                nc.sync.dma_start(m_sb[:, 0, h], mp_d[:, h])

            # bias: 4 KB DMA + on-chip broadcast (gpsimd is otherwise idle)
            bias_row = consts.tile([1, D], F32)
            nc.sync.dma_start(bias_row[:], b_d[:])
            bias_sb = consts.tile([128, D], F32)
            nc.gpsimd.partition_broadcast(bias_sb[:], bias_row[:])

            def rhs_view(k, kb):
                sb, half = k // 2, k % 2
                sidx = 0 if SGN[kb][sb] > 0 else 1
                return m_sb[:, sidx, half, Q_IDX[kb][sb], :]

            # PE warm-up absorbs the identity producer dep so the first
            # transpose carries only its x-DMA wait (ONE wait per Matmult).
            warm_a = psum_mm.tile([128, 128], F32R, tag="mm_ps")
            nc.tensor.transpose(warm_a[:], identity[:], identity[:])
            # dummy matmuls fill the initial DMA wait so the HAM clock-gate
            # is open when real matmuls start (transposes don't count as
            # PE-busy for HAM)
            for _w in range(12):
                warm_m = psum_mm.tile([128, 128], F32, tag="mm_ps", name=f"warm_m{_w}")
                nc.tensor.matmul(warm_m[:], identity[:], identity[:], start=True, stop=True)

            # Warm-up transposes that absorb each weight-pack DMA wait,
            # emitted just before the block-0 matmul that first needs it.
            warm_before = {0: [(0, 0)], 1: [(0, 1)], 2: [(1, 0)], 3: [(1, 1)]}

            xt_tiles = {}
            # Software pipeline: stage A (DMA + transpose + evacuate) runs
            # one block ahead of stage B (matmuls + bias-add + store).
            for blk in range(BLOCKS + 1):
                if blk < BLOCKS:
                    rows = slice(blk * 128, (blk + 1) * 128)
                    if blk in early_x:
                        x_sb = early_x.pop(blk)
                    else:
                        x_sb = xin_pool.tile([128, D], F32R, tag="x_sb")
                        nc.sync.dma_start(x_sb[:], x_d[rows, :])
                    xt_ps = psum_t.tile([128, D], F32R, tag="xt_ps")
                    for k in range(KT):
                        nc.tensor.transpose(
                            xt_ps[:, k * 128 : (k + 1) * 128],
                            x_sb[:, k * 128 : (k + 1) * 128],
                            identity[:],
                        )
                    xt_sb = xt_pool.tile([128, D], F32R, tag="xt_sb")
                    if blk == 0:
                        nc.vector.tensor_copy(out=xt_sb[:, 0:512], in_=xt_ps[:, 0:512])
                        nc.vector.tensor_copy(out=xt_sb[:, 512:D], in_=xt_ps[:, 512:D])
                    else:
                        nc.vector.tensor_copy(out=xt_sb[:], in_=xt_ps[:])
                    xt_tiles[blk] = xt_sb
                    if blk == 0:
                        # negate the weight pack halves as they arrive
                        for h in (0, 1):
                            nc.vector.tensor_scalar_mul(
                                m_sb[:, 1, h], m_sb[:, 0, h], -1.0
                            )

                if blk >= 1:
                    mblk = blk - 1
                    rows = slice(mblk * 128, (mblk + 1) * 128)
                    xt_sb = xt_tiles.pop(mblk)
                    out_sb = out_pool.tile([128, D], F32, tag="out_sb")
                    mm_ps = [
                        psum_mm.tile(
                            [128, 512], F32, tag="mm_ps", name=f"mm_ps_{mblk}_{n}"
                        )
                        for n in range(2)
                    ]
                    # k-outer: each stationary xt slice loads once for all
                    # four 256-column output slabs.
                    last = mblk == BLOCKS - 1
                    if last:
                        # bank-outer: close bank 0 halfway through so its
                        # bias-add + store overlap bank 1's matmuls
                        order = [(k, kb) for n_ in range(2) for k in range(KT) for kb in (2 * n_, 2 * n_ + 1)]
                    else:
                        order = [(k, kb) for k in range(KT) for kb in range(4)]
                    for k, kb in order:
                        if mblk == 0 and kb == 0:
                            for s, h in warm_before.get(k, []):
                                warm_k = psum_mm.tile(
                                    [128, 128], F32R, tag="mm_ps", name=f"warm_{s}{h}"
                                )
                                nc.tensor.transpose(
                                    warm_k[:], m_sb[:, s, h, 0, 0:128], identity[:]
                                )
                        n, c = kb // 2, kb % 2
                        nc.tensor.matmul(
                            mm_ps[n][:, c * 256 : (c + 1) * 256],
                            xt_sb[:, k * 128 : (k + 1) * 128],
                            rhs_view(k, kb),
                            start=(k == 0 and c == 0),
                            stop=(k == KT - 1 and c == 1),
                            skip_group_check=True,
                        )
                    if mblk == 0:
                        # DVE warm-up observes the bias DMA queue before the
                        # first add so the add itself carries one wait.
                        warm_v = consts.tile([128, 1], F32)
                        nc.vector.tensor_copy(out=warm_v[:], in_=bias_sb[:, 0:1])
                    for n in range(2):
                        nc.vector.tensor_add(
                            out=out_sb[:, n * 512 : (n + 1) * 512],
                            in0=mm_ps[n][:],
                            in1=bias_sb[:, n * 512 : (n + 1) * 512],
                        )
                        eng = nc.sync if (last and n == 1) else nc.scalar
                        eng.dma_start(
                            o_d[rows, n * 512 : (n + 1) * 512],
                            out_sb[:, n * 512 : (n + 1) * 512],
                        )
    nc.compile()
    return nc


def kernel(x, A_stack, B_stack, bias):
    from concourse.bass_utils import run_bass_kernel_spmd

    global _cached_nc
    x = np.ascontiguousarray(np.asarray(x, dtype=np.float32))
    A_stack = np.asarray(A_stack, dtype=np.float32)
    B_stack = np.asarray(B_stack, dtype=np.float32)
    bias = np.asarray(bias, dtype=np.float32)

    # M_q[(sr,i),(kr,j)] = sum_r A[r,q,kr,sr] * B[r,j,i]; W block (sb,kb)
    # = SGN[kb][sb] * M[Q[kb][sb]] reproduces W[si,kj] = sum_r H B.
    M = np.einsum("rqks,rji->qsikj", A_stack, B_stack).reshape(4, 256, 256)
    mpack = np.empty((128, 2, 4, 256), dtype=np.float32)
    for h in range(2):
        mpack[:, h] = np.moveaxis(M[:, h * 128 : (h + 1) * 128, :], 0, 1)
    mpack = np.ascontiguousarray(mpack)

    bias_b = np.ascontiguousarray(bias[None, :])

    shards = x.reshape(N_CORES, TOK_PER_CORE, D)
    if _cached_nc is None:
        _cached_nc = _build()
    in_maps = [
        {"x": shards[c], "mpack": mpack, "biasb": bias_b}
        for c in range(N_CORES)
    ]
    res = run_bass_kernel_spmd(
        _cached_nc, in_maps, core_ids=list(range(N_CORES)), trace=False
    )
    out = np.concatenate([r["out"] for r in res.results], axis=0)
    return out.reshape(B, T, D)


# revision 32
# speedup vs baseline: 1.0016x; 1.0016x over previous
"""Trainium2 Bass kernel for nn_BalancedHamiltonLayer.

The reference computes, per token-matrix X_n (32x32 view of each 1024-dim
token):  out_n = sum_r H_r @ X_n @ B_r^T  + bias, with H_r the 32x32 Hamilton
matrix of A_stack[r].  That is a fixed linear map on the flattened token:

    out[n, k*32+j] = sum_{s,i} W[s*32+i, k*32+j] * x[n, s*32+i]
    W[si, kj]      = sum_r H[r,k,s] * B[r,j,i]          (1024x1024, fp32)

so the whole layer is one dense (12288, 1024) @ (1024, 1024) matmul + bias.
The tiny factor stacks are folded on the host; x is sharded along the batch
axis across the 8 NeuronCores (data parallel, no collectives).

W's 4x4 grid of 256x256 blocks are +-copies of only FOUR unique matrices
M_q[(sr,i),(kr,j)] = sum_r A[r,q,kr,sr]*B[r,j,i] (quaternion structure), so
the host ships 2 MB of +-M instead of 4 MB of W, and every matmul rhs is a
(128, 256) view straight into that pack - no on-chip weight expansion.

Per core (1536 tokens, 12 blocks of 128), software-pipelined:
  natural-layout x DMA -> PE transpose (fp32r, via identity) into PSUM ->
  one DVE evacuation per block -> 32 fp32r matmuls (K=8x128, N=4x256)
  accumulating in PSUM -> DVE bias-add -> DMA out (scalar-engine HWDGE).
Transposes are emitted one block ahead of the matmuls so the DVE evacuation
overlaps the previous block's matmuls instead of stalling the PE.

PSUM bank accumulation: only the FIRST matmul touching a bank per block uses
start=True (which clears the whole bank's has_written bits); the second
256-column slice then overwrites-where-clear, and all k>0 matmuls
accumulate.  fp32r (rounded fp32) streams at 1 cycle/row for N>=256 (4x
faster than fp32) at ~1.5e-4 relative error.

Self-loading 4-byte Matmults only fit ONE sync wait in the S3_LW ISA
struct, so the kernel keeps every Matmult at <=1 wait: PE warm-up
transposes (interleaved with block 0's matmuls) absorb the weight-pack DMA
deps, and all PSUM evacuations go through the vector engine so a single
DVE-sem wait on the first matmul of each block transitively covers every
WAR hazard.
"""

import numpy as np

B, T, D = 48, 256, 1024
N_CORES = 8
TOK = B * T                     # 12288 tokens
TOK_PER_CORE = TOK // N_CORES   # 1536
BLOCKS = TOK_PER_CORE // 128    # 12
KT = D // 128                   # 8 contraction tiles

# Quaternion block structure: W block (sb, kb) = SGN[kb][sb] * M[Q[kb][sb]]
Q_IDX = [[0, 1, 2, 3], [1, 0, 3, 2], [2, 3, 0, 1], [3, 2, 1, 0]]
SGN = [[1, -1, -1, -1], [1, 1, -1, 1], [1, 1, 1, -1], [1, -1, 1, 1]]

_cached_nc = None


def _build():
    import concourse.bacc as bacc
    import concourse.mybir as mybir
    import concourse.tile as tile

    F32R = mybir.dt.float32r
    F32 = mybir.dt.float32

    nc = bacc.Bacc("TRN2", target_bir_lowering=False)
    x_d = nc.dram_tensor("x", [TOK_PER_CORE, D], F32R, kind="ExternalInput")
    mp_d = nc.dram_tensor("mpack", [128, 2, 4, 256], F32R, kind="ExternalInput")
    b_d = nc.dram_tensor("biasb", [1, D], F32, kind="ExternalInput")
    o_d = nc.dram_tensor("out", [TOK_PER_CORE, D], F32, kind="ExternalOutput")

    with tile.TileContext(nc) as tc:
        with (
            tc.tile_pool(name="consts", bufs=1) as consts,
            tc.tile_pool(name="xin", bufs=6) as xin_pool,
            tc.tile_pool(name="xt", bufs=6) as xt_pool,
            tc.tile_pool(name="outp", bufs=BLOCKS) as out_pool,
            tc.tile_pool(name="psum_t", bufs=2, space="PSUM") as psum_t,
            tc.tile_pool(name="psum_mm", bufs=4, space="PSUM") as psum_mm,
        ):
            # identity built on-chip: gpsimd writes f32, DVE cast-copies to
            # f32r (the canonical "round to fp32r" producer) - no DMA.
            from concourse.masks import make_identity

            identity_f32 = consts.tile([128, 128], F32)
            make_identity(nc, identity_f32)
            identity = consts.tile([128, 128], F32R)
            nc.vector.tensor_copy(out=identity[:], in_=identity_f32[:])

            m_sb = consts.tile([128, 2, 2, 4, 256], F32R)
            # Prefetch the first x blocks ahead of the weight pack so the PE
            # has transpose work while the weights stream in.
            early_x = {}
            for blk in range(2):
                x_sb = xin_pool.tile([128, D], F32R, tag="x_sb", name=f"x_sb_{blk}")
                if blk == 0:
                    nc.sync.dma_start(x_sb[:, 0:512], x_d[0:128, 0:512])
                    nc.sync.dma_start(x_sb[:, 512:D], x_d[0:128, 512:D])
                else:
                    nc.sync.dma_start(x_sb[:], x_d[blk * 128 : (blk + 1) * 128, :])
                early_x[blk] = x_sb


            # Weight pack DMAs: +M only (1 MB); -M is produced on the DVE
            # during block 0 (cheaper than 1 MB more of front-loaded DMA).
            for h in (0, 1):
                nc.sync.dma_start(m_sb[:, 0, h], mp_d[:, h])

            # bias: 4 KB DMA + on-chip broadcast (gpsimd is otherwise idle)
            bias_row = consts.tile([1, D], F32)
            nc.sync.dma_start(bias_row[:], b_d[:])
            bias_sb = consts.tile([128, D], F32)
            nc.gpsimd.partition_broadcast(bias_sb[:], bias_row[:])

            def rhs_view(k, kb):
                sb, half = k // 2, k % 2
                sidx = 0 if SGN[kb][sb] > 0 else 1
                return m_sb[:, sidx, half, Q_IDX[kb][sb], :]

            def block_mm_items(k):
                """(n, c, rhs) triples covering kb 0..3 of chunk k.  For
                sb=0 and sb=2 the kb-pairs are adjacent same-sign q-slabs in
                the pack, so they merge into N=512 matmuls (c=None) - same
                PE cycles, 25% fewer self-weight-loads."""
                sb, half = k // 2, k % 2
                if sb == 0:
                    return [
                        (0, None, m_sb[:, 0, half, 0:2, :]),
                        (1, None, m_sb[:, 0, half, 2:4, :]),
                    ]
                if sb == 2:
                    return [
                        (0, None, m_sb[:, 1, half, 2:4, :]),
                        (1, None, m_sb[:, 0, half, 0:2, :]),
                    ]
                return [
                    (kb // 2, kb % 2, rhs_view(k, kb)) for kb in range(4)
                ]

            # PE warm-up absorbs the identity producer dep so the first
            # transpose carries only its x-DMA wait (ONE wait per Matmult).
            warm_a = psum_mm.tile([128, 128], F32R, tag="mm_ps")
            nc.tensor.transpose(warm_a[:], identity[:], identity[:])
            # dummy matmuls fill the initial DMA wait so the HAM clock-gate
            # is open when real matmuls start (transposes don't count as
            # PE-busy for HAM)
            for _w in range(12):
                warm_m = psum_mm.tile([128, 128], F32, tag="mm_ps", name=f"warm_m{_w}")
                nc.tensor.matmul(warm_m[:], identity[:], identity[:], start=True, stop=True)

            # Warm-up transposes that absorb each weight-pack DMA wait,
            # emitted just before the block-0 matmul that first needs it.
            warm_before = {0: [(0, 0)], 1: [(0, 1)], 2: [(1, 0)], 3: [(1, 1)]}

            xt_tiles = {}
            # Software pipeline: stage A (DMA + transpose + evacuate) runs
            # one block ahead of stage B (matmuls + bias-add + store).
            for blk in range(BLOCKS + 1):
                if blk < BLOCKS:
                    rows = slice(blk * 128, (blk + 1) * 128)
                    if blk in early_x:
                        x_sb = early_x.pop(blk)
                    else:
                        x_sb = xin_pool.tile([128, D], F32R, tag="x_sb")
                        nc.sync.dma_start(x_sb[:], x_d[rows, :])
                    xt_ps = psum_t.tile([128, D], F32R, tag="xt_ps")
                    for k in range(KT):
                        nc.tensor.transpose(
                            xt_ps[:, k * 128 : (k + 1) * 128],
                            x_sb[:, k * 128 : (k + 1) * 128],
                            identity[:],
                        )
                    xt_sb = xt_pool.tile([128, D], F32R, tag="xt_sb")
                    if blk == 0:
                        nc.vector.tensor_copy(out=xt_sb[:, 0:512], in_=xt_ps[:, 0:512])
                        nc.vector.tensor_copy(out=xt_sb[:, 512:D], in_=xt_ps[:, 512:D])
                    else:
                        nc.vector.tensor_copy(out=xt_sb[:], in_=xt_ps[:])
                    xt_tiles[blk] = xt_sb
                    if blk == 0:
                        # negate the weight pack halves as they arrive
                        for h in (0, 1):
                            nc.vector.tensor_scalar_mul(
                                m_sb[:, 1, h], m_sb[:, 0, h], -1.0
                            )

                if blk >= 1:
                    mblk = blk - 1
                    rows = slice(mblk * 128, (mblk + 1) * 128)
                    xt_sb = xt_tiles.pop(mblk)
                    out_sb = out_pool.tile([128, D], F32, tag="out_sb")
                    mm_ps = [
                        psum_mm.tile(
                            [128, 512], F32, tag="mm_ps", name=f"mm_ps_{mblk}_{n}"
                        )
                        for n in range(2)
                    ]
                    # k-outer: each stationary xt slice loads once for all
                    # of its output slabs.
                    last = mblk == BLOCKS - 1
                    items = [
                        (k, n, c, rhs)
                        for k in range(KT)
                        for (n, c, rhs) in block_mm_items(k)
                    ]
                    if last:
                        # bank-outer: close bank 0 halfway through so its
                        # bias-add + store overlap bank 1's matmuls
                        items.sort(key=lambda t: t[1])
                    seen_k = set()
                    for k, n, c, rhs in items:
                        if mblk == 0 and k not in seen_k:
                            seen_k.add(k)
                            for s, h in warm_before.get(k, []):
                                warm_k = psum_mm.tile(
                                    [128, 128], F32R, tag="mm_ps", name=f"warm_{s}{h}"
                                )
                                nc.tensor.transpose(
                                    warm_k[:], m_sb[:, s, h, 0, 0:128], identity[:]
                                )
                        dst = (
                            mm_ps[n][:]
                            if c is None
                            else mm_ps[n][:, c * 256 : (c + 1) * 256]
                        )
                        nc.tensor.matmul(
                            dst,
                            xt_sb[:, k * 128 : (k + 1) * 128],
                            rhs,
                            start=(k == 0),
                            stop=(k == KT - 1 and c == 1),
                            skip_group_check=True,
                        )
                    if mblk == 0:
                        # DVE warm-up observes the bias DMA queue before the
                        # first add so the add itself carries one wait.
                        warm_v = consts.tile([128, 1], F32)
                        nc.vector.tensor_copy(out=warm_v[:], in_=bias_sb[:, 0:1])
                    for n in range(2):
                        nc.vector.tensor_add(
                            out=out_sb[:, n * 512 : (n + 1) * 512],
                            in0=mm_ps[n][:],
                            in1=bias_sb[:, n * 512 : (n + 1) * 512],
                        )
                        eng = nc.sync if (last and n == 1) else nc.scalar
                        eng.dma_start(
                            o_d[rows, n * 512 : (n + 1) * 512],
                            out_sb[:, n * 512 : (n + 1) * 512],
                        )
    nc.compile()
    return nc


def kernel(x, A_stack, B_stack, bias):
    from concourse.bass_utils import run_bass_kernel_spmd

    global _cached_nc
    x = np.ascontiguousarray(np.asarray(x, dtype=np.float32))
    A_stack = np.asarray(A_stack, dtype=np.float32)
    B_stack = np.asarray(B_stack, dtype=np.float32)
    bias = np.asarray(bias, dtype=np.float32)

    # M_q[(sr,i),(kr,j)] = sum_r A[r,q,kr,sr] * B[r,j,i]; W block (sb,kb)
    # = SGN[kb][sb] * M[Q[kb][sb]] reproduces W[si,kj] = sum_r H B.
    M = np.einsum("rqks,rji->qsikj", A_stack, B_stack).reshape(4, 256, 256)
    mpack = np.empty((128, 2, 4, 256), dtype=np.float32)
    for h in range(2):
        mpack[:, h] = np.moveaxis(M[:, h * 128 : (h + 1) * 128, :], 0, 1)
    mpack = np.ascontiguousarray(mpack)

    bias_b = np.ascontiguousarray(bias[None, :])

    shards = x.reshape(N_CORES, TOK_PER_CORE, D)
    if _cached_nc is None:
        _cached_nc = _build()
    in_maps = [
        {"x": shards[c], "mpack": mpack, "biasb": bias_b}
        for c in range(N_CORES)
    ]
    res = run_bass_kernel_spmd(
        _cached_nc, in_maps, core_ids=list(range(N_CORES)), trace=False
    )
    out = np.concatenate([r["out"] for r in res.results], axis=0)
    return out.reshape(B, T, D)
